# revision 42
# baseline (speedup 1.0000x reference)
"""Multi-head attention (B=4, N=2048, C=1024, H=16) on 8 TRN2 NeuronCores.

Sharding: core = 2*b + half handles batch b, heads half*8 .. half*8+7.
Each core computes QKV for its 8 heads, full attention for them, and a
partial projection (its 512 rows of W_proj). Host sums the two partials
per batch and adds the bias.

All matmul operands are fp16 (1 cycle/row on the PE vs 2 for f32r, same
~11-bit mantissa); accumulation stays fp32 in PSUM. The host pre-casts
to fp16, pre-transposes x, and pre-arranges every weight into its
on-chip SBUF layout so each weight DMA is one contiguous transfer.

On-chip layout is "transposed": Q^T/K^T [d, n] come straight out of the
QKV matmuls (lhsT = W chunk, rhs = x^T), scores are computed as
S^T[m, n] so that exp(S^T) = P^T is directly the moving operand of the
AV matmul (V chunk stationary). exp is shifted by a constant bias (it
cancels in softmax) to keep P in fp16 range. Row sums of P ride along as
a 65th stationary column of ones.

The kernel is paced by ScalarE (256 exp tiles of [128,1024] ~= 284us).
Scheduling keeps ScalarE saturated:
 - The scores->exp chain is emitted at high scheduler priority; a deep
   pt ring lets the AV/projection backlog lag behind exp (the V
   projection streamed into C(0) nb=0 overloads the PE locally).
 - x^T DMAs in 16 pieces ordered so the first QK units start ~5us in.
 - Each pair's QK units stream into the PREVIOUS pair's loop only for
   k0/q0; the rest stream into the pair's own first two nb-blocks.
 - Half of the output projection interleaves into pair 3's last two
   nb-blocks.
 - Normalization: raw row sums broadcast via K=1 matmul FIRST, then
   reciprocal_approx_fast (DVE) off the PE critical path.
"""

import functools
from collections import defaultdict
from contextlib import ExitStack

import numpy as np

import concourse.bass as bass
import concourse.tile as tile
from concourse import bacc, mybir
from concourse import dve_ops as _do
from concourse.bass_utils import run_bass_kernel_spmd
from concourse.dve_ops import DveOp, has_src1, lower
from concourse.dve_spec import C0, C1, C2, Spec, Src0, sq
from concourse.dve_table_gen import dve_ver_for
from concourse.dve_uop import DveOpSpec

F32 = mybir.dt.float32
F16 = mybir.dt.float16
AF = mybir.ActivationFunctionType

B, N, C = 4, 2048, 1024
H, D = 16, 64
P = 128
NCORES = 8
HPC = 8            # heads per core
PAIRS = HPC // 2   # 4
DCORE = HPC * D    # 512 attention columns per core
SCALE = float(H) ** -0.5  # 0.25 (faithful to reference: num_heads**-0.5)
EXP_BIAS = -5.0    # exp(scale*s + bias): cancels in softmax, keeps fp16 range
NB = N // 512      # 4 n blocks
NT = N // P        # 16 tiles of 128
CT = C // P        # 8 contraction chunks
VW = D + 1         # V columns per head incl. the ones column (row sums)
MBLK = HPC * VW    # 520 v_sb columns per m-tile

SPRIO = 4000       # priority boost for the scores->exp chain
KPRIO = 384        # priority boost for deadline-critical QK units
AVPRIO = 128       # priority boost for AV matmuls (above interleave units)
DVE_EXP = False    # offload some exp tiles to the DVE (see EXP32_* ops)

LAST_RESULT = None  # BassKernelResults of the most recent run (for test.py)

# --- custom-DVE exp: offload part of the softmax exp from ScalarE -------- #
# exp(SCALE*s + EXP_BIAS) = h(s)^32 with h a general cubic fitted to
# 2^((SCALE*s + EXP_BIAS)*log2e/32) over s in [-20, 48] (the occupied
# score range; beyond it only negligible softmax weights are affected).
# Two single-uop DVE ops: cubic (6 ALUs), then (+d) and 5 squarings
# (6 ALUs). Max rel err ~1.5e-3 where softmax weights matter -- same
# order as the fp16 P quantization already in use.
EXPA = 0.004227742419273297    # h = (EXPA*s + EXPB)^3 + EXPC*s  (op1)
EXPB = 0.48705636560136756
EXPC = 0.0036706996190657734
EXPD = 0.7398024337468856      # out = (h + EXPD)^32             (op2)

_t = Src0 * C0 + C1
EXP32_CUBIC = DveOp(
    "EXP32_CUBIC",
    Spec(
        body=sq(_t) * _t + Src0 * C2,
        reference=lambda in0, in1, s0, s1, imm2: (
            (in0.astype(np.float32) * s0 + s1) ** 3 + in0 * imm2
        ).astype(np.float32),
    ),
    subdim=False,
    uops_sha={},
)
EXP32_SQ5 = DveOp(
    "EXP32_SQ5",
    Spec(
        body=sq(sq(sq(sq(sq(Src0 + C0))))),
        reference=lambda in0, in1, s0, s1, imm2: (
            (in0.astype(np.float32) + s0) ** 32
        ).astype(np.float32),
    ),
    subdim=False,
    uops_sha={},
)


def _register_dve_ops():
    ver = dve_ver_for("TRN2")
    for op in (EXP32_CUBIC, EXP32_SQ5):
        if op.name in _do._SUB_OPCODE_FOR_NAME:
            continue
        _do.OPS.append(op)
        _do.CUSTOM_DVE_SPECS[op.name] = op.spec
        _do._SUB_OPCODE_FOR_NAME[op.name] = \
            _do._CUSTOM_DVE_ROW_BASE + len(_do.OPS) - 1
        spec_c = DveOpSpec(
            name=op.name,
            opcode=_do.get_dve_sub_opcode(op.name),
            uops=lower(op.spec, ver=ver),
            rd1_en=has_src1(op.spec),
        )
        op.uops_sha[ver] = spec_c.sha(ver)


_register_dve_ops()


def _kernel_body(tc, out_d, xt_d, wq_d, wk_d, wv_d, wp_d):
    nc = tc.nc
    with ExitStack() as ctx:
        const = ctx.enter_context(tc.tile_pool(name="const", bufs=1))
        ones_f = const.tile([P, P], F32)
        nc.vector.memset(ones_f, 1.0)
        ones_bc = const.tile([P, 64], F16)
        nc.vector.tensor_copy(ones_bc, ones_f[:, 0:64])
        ebias = const.tile([P, 1], F32)
        nc.vector.memset(ebias, EXP_BIAS)

        # attT: pair p occupies cols [p*N, (p+1)*N); partitions = 2 heads x 64
        attT_pool = ctx.enter_context(tc.tile_pool(name="attT", bufs=1))
        attT = attT_pool.tile([P, PAIRS * N], F16)

        # PSUM budget (8 banks): ps_s 2x[128,1024]=4, ps_av (AV accums,
        # tags avA/avB)=2, ps_w (B1 psv / B2 psq / D psp / sum-bc)=2.
        ps_s = ctx.enter_context(tc.tile_pool(name="ps_s", bufs=2, space="PSUM"))
        ps_av = ctx.enter_context(tc.tile_pool(name="ps_av", bufs=1, space="PSUM"))
        ps_w = ctx.enter_context(tc.tile_pool(name="ps_w", bufs=2, space="PSUM"))

        xt_pool = ctx.enter_context(tc.tile_pool(name="xt", bufs=1))
        xt = xt_pool.tile([P, CT * N], F16)
        v_pool = ctx.enter_context(tc.tile_pool(name="v", bufs=1))
        v_sb = v_pool.tile([P, NT * MBLK], F16)
        wv_pool = ctx.enter_context(tc.tile_pool(name="wv", bufs=1))
        wv_sb = wv_pool.tile([P, CT * DCORE], F16)
        wp_pool = ctx.enter_context(tc.tile_pool(name="wp", bufs=1))
        stage_pool = ctx.enter_context(tc.tile_pool(name="stage", bufs=3))
        qt_pool = ctx.enter_context(tc.tile_pool(name="qt", bufs=2))
        kt_pool = ctx.enter_context(tc.tile_pool(name="kt", bufs=2))
        wqk_pool = ctx.enter_context(tc.tile_pool(name="wqk", bufs=3))
        pt_pool = ctx.enter_context(tc.tile_pool(name="pt", bufs=30))
        rs_pool = ctx.enter_context(tc.tile_pool(name="rs", bufs=2))
        rb_pool = ctx.enter_context(tc.tile_pool(name="rb", bufs=2))
        sc_pool = ctx.enter_context(tc.tile_pool(name="sc", bufs=2))

        # V layout: m-tile m at cols [m*MBLK, ...); head hl at
        # [m*MBLK + hl*VW, +D], then a ones column (for row sums)
        ones_cols = v_sb.rearrange("q (g k) -> q g k", k=VW)[:, :, D:VW]
        nc.vector.tensor_copy(
            ones_cols, ones_f.rearrange("q (g k) -> q g k", k=1))

        def xt_dma(lo, width):
            for j in range(CT):
                nc.sync.dma_start(
                    out=xt[:, j * N + lo: j * N + lo + width],
                    in_=xt_d[j * P:(j + 1) * P, lo: lo + width])

        def b1_chunk(m):
            """V for all 8 heads, one m-tile: 8 MMs + strided evict."""
            psv = ps_w.tile([P, DCORE], F32, tag="w")
            for cc in range(CT):
                nc.tensor.matmul(
                    psv,
                    xt[:, cc * N + m * P: cc * N + (m + 1) * P],
                    wv_sb[:, cc * DCORE:(cc + 1) * DCORE],
                    start=(cc == 0), stop=(cc == CT - 1))
            nc.vector.tensor_copy(
                v_sb[:, m * MBLK:(m + 1) * MBLK].rearrange(
                    "q (h k) -> q h k", k=VW)[:, :, 0:D],
                psv.rearrange("q (h k) -> q h k", k=D))

        qkts = {}

        def b2_units(p):
            """QK units for pair p: dict of closures. Each mm unit is
            8 accumulating MMs + a psq evict for one 512-col block."""
            qt = qt_pool.tile([P, N], F16, tag="qt")
            kt = kt_pool.tile([P, N], F16, tag="kt")
            qkts[p] = (qt, kt)
            units = {}
            for key, w_d, dst in (("q", wq_d, qt), ("k", wk_d, kt)):
                wt = wqk_pool.tile([P, CT * P], F16, tag="w")

                def dma(w_d=w_d, wt=wt):
                    nc.sync.dma_start(out=wt, in_=w_d[p * P:(p + 1) * P, :])
                units["dma_" + key] = dma
                for nb in range(NB):
                    def mmq(wt=wt, dst=dst, nb=nb):
                        psq = ps_w.tile([P, 512], F32, tag="w")
                        for cc in range(CT):
                            nc.tensor.matmul(
                                psq,
                                wt[:, cc * P:(cc + 1) * P],
                                xt[:, cc * N + nb * 512:
                                   cc * N + nb * 512 + 512],
                                start=(cc == 0), stop=(cc == CT - 1))
                        nc.vector.tensor_copy(
                            dst[:, nb * 512:(nb + 1) * 512], psq)
                    units[key + str(nb)] = mmq
            return units

        def kp(u):
            def run():
                with tc.high_priority(offset=KPRIO):
                    u()
            return run

        wp_sb = wp_pool.tile([P, PAIRS * C], F16)

        def d_unit(i, co):
            """One projection output tile: 4 accumulating MMs + evict + DMA."""
            psp = ps_w.tile([P, 512], F32, tag="w")
            for dc in range(PAIRS):
                nc.tensor.matmul(
                    psp,
                    attT[:, dc * N + i * P: dc * N + (i + 1) * P],
                    wp_sb[:, dc * C + co * 512: dc * C + co * 512 + 512],
                    start=(dc == 0), stop=(dc == PAIRS - 1))
            st = stage_pool.tile([P, 512], F32, tag="st")
            nc.vector.tensor_copy(st, psp)
            nc.sync.dma_start(
                out=out_d[i * P:(i + 1) * P, co * 512: co * 512 + 512],
                in_=st)

        # ---- Lead-in. DMA order matters: pair-0 QK weights, then the xt
        # blocks in consumption order (V weights late -- B1 deadlines are
        # soft). Junk matmuls on uninitialized SBUF keep the PE's HAM
        # activity monitor warm so the first real matmuls run at 2.4GHz.
        u0 = b2_units(0)
        u0["dma_k"]()
        u0["dma_q"]()
        xt_dma(0, 512)
        xt_dma(512, 512)
        nc.sync.dma_start(out=wv_sb, in_=wv_d)
        xt_dma(1024, 1024)
        wu = ps_w.tile([P, 512], F32, tag="w")
        for _ in range(20):
            nc.tensor.matmul(wu[0:64, 0:512], ones_bc, attT[:, 0:512],
                             start=True, stop=True, skip_group_check=True)
        u0["k0"]()
        u0["q0"]()
        b1_chunk(0)
        b1_chunk(1)

        pending = {0: u0}
        for p in range(PAIRS):
            qt, kt = qkts.pop(p)
            imap = defaultdict(list)
            # this pair's remaining QK units stream into its first two
            # nb-blocks (k-units are deadline-critical for the S chain).
            # Pair 3 processes nb in order [1,2,3,0] and had q1 (not q0)
            # prefetched, so its q-units stream in that order too.
            un = pending.pop(p)
            imap[(0, 0)].append(kp(un["k1"]))
            imap[(0, 4)].append(kp(un["k2"]))
            imap[(0, 8)].append(kp(un["k3"]))
            qseq = ("q1", "q2", "q3") if p < PAIRS - 1 else ("q2", "q3", "q0")
            imap[(0, 12)].append(kp(un[qseq[0]]))
            imap[(1, 1)].append(kp(un[qseq[1]]))
            imap[(1, 5)].append(kp(un[qseq[2]]))
            if p == 0:
                # stream remaining V tiles into C(0) nb=0
                for m in range(2, NT):
                    imap[(0, m - 2)].append(functools.partial(b1_chunk, m))
            if p + 1 < PAIRS:
                # prefetch must allocate its ps_w tiles BEFORE the nb=2
                # tail's bc tiles, or the 2-deep ring traps the kt evict
                # behind the nb=3 normalization chain (7us boundary stall)
                nxt = b2_units(p + 1)
                pending[p + 1] = nxt
                imap[(2, 0)].append(nxt["dma_k"])
                imap[(2, 2)].append(nxt["dma_q"])
                imap[(2, 4)].append(kp(nxt["k0"]))
                # pair 3 starts with nb=1, so prefetch its q1 instead
                imap[(2, 10)].append(
                    kp(nxt["q0"] if p + 1 < PAIRS - 1 else nxt["q1"]))
            else:
                # pair 3: wp DMA + interleave projection tiles. The nb
                # order is rotated to [1,2,3,0] so only the i<4 tiles
                # (gated on nb=0, processed last) remain for the tail.
                nc.sync.dma_start(out=wp_sb, in_=wp_d)
                for j in range(4):
                    for co in range(2):
                        imap[(1, 1 + 2 * (2 * j + co))].append(
                            functools.partial(d_unit, j + 4, co))
                        imap[(2, 1 + 2 * (2 * j + co))].append(
                            functools.partial(d_unit, j + 8, co))
                        imap[(3, 1 + 2 * (2 * j + co))].append(
                            functools.partial(d_unit, j + 12, co))

            nb_order = (1, 2, 3, 0) if p == PAIRS - 1 else (0, 1, 2, 3)
            for pos in range(NB):
                nb = nb_order[pos]
                nsl = slice(nb * 512, nb * 512 + 512)
                osl = slice(p * N + nb * 512, p * N + nb * 512 + 512)
                ps_av_a = ps_av.tile([P, 512], F32, tag="avA")
                ps_av_b = ps_av.tile([P, 512], F32, tag="avB")
                for m in range(NT):
                    first = (m == 0)
                    last = (m == NT - 1)
                    # A few tiles per pair run their exp on the DVE using
                    # ps_w scratch banks instead of the ps_s ring, so the
                    # ScalarE pipeline (paced by that ring) never notices.
                    # Only in the quiet nb-slot-3 blocks of pairs 0-2
                    # where ps_w has free slots.
                    off = DVE_EXP and pos == 3 and p < PAIRS - 1 \
                        and m in (2, 6, 10)
                    with tc.high_priority(offset=SPRIO):
                        pt = pt_pool.tile([P, 1024], F16, tag="pt")
                        if off:
                            sa = ps_w.tile([P, 512], F32, tag="w")
                            sb = ps_w.tile([P, 512], F32, tag="w")
                            nc.tensor.matmul(
                                sa, kt[0:64, m * P:(m + 1) * P],
                                qt[0:64, nsl], start=True, stop=True)
                            nc.tensor.matmul(
                                sb, kt[64:128, m * P:(m + 1) * P],
                                qt[64:128, nsl], start=True, stop=True)
                            sc = sc_pool.tile([P, 1024], F32, tag="sc")
                            nc.vector._custom_dve(
                                EXP32_CUBIC, out=sc[:, 0:512], in0=sa,
                                s0=EXPA, s1=EXPB, imm2=EXPC)
                            nc.vector._custom_dve(
                                EXP32_CUBIC, out=sc[:, 512:1024], in0=sb,
                                s0=EXPA, s1=EXPB, imm2=EXPC)
                            nc.vector._custom_dve(
                                EXP32_SQ5, out=pt, in0=sc, s0=EXPD)
                        else:
                            ps_s_t = ps_s.tile([P, 1024], F32, tag="s")
                            # scores^T chunk [m-tile, n-block]; two heads
                            # as concurrent K=64 row-tiles (0,0)/(64,0)
                            nc.tensor.matmul(
                                ps_s_t[:, 0:512],
                                kt[0:64, m * P:(m + 1) * P],
                                qt[0:64, nsl],
                                start=True, stop=True)
                            nc.tensor.matmul(
                                ps_s_t[:, 512:1024],
                                kt[64:128, m * P:(m + 1) * P],
                                qt[64:128, nsl],
                                start=True, stop=True)
                            nc.scalar.activation(pt, ps_s_t, AF.Exp,
                                                 scale=SCALE, bias=ebias)
                    for u in imap.get((pos, m), ()):
                        u()
                    # AV with fused row-sums: lhsT = [V_h | 1] (M = 65);
                    # partition 64 accumulates the softmax denominators
                    vbase = m * MBLK + 2 * p * VW
                    nc.tensor.matmul(
                        ps_av_a[0:VW, :],
                        v_sb[:, vbase: vbase + VW],
                        pt[:, 0:512],
                        start=first, stop=last, skip_group_check=True)
                    nc.tensor.matmul(
                        ps_av_b[0:VW, :],
                        v_sb[:, vbase + VW: vbase + 2 * VW],
                        pt[:, 512:1024],
                        start=first, stop=last, skip_group_check=True)
                # Evict raw sums + unnormalized AV rows; broadcast the
                # RAW sums via K=1 matmul (PE never waits on a recip),
                # then fast-reciprocal the broadcast tile and multiply.
                rs = rs_pool.tile([P, 1024], F16, tag="rs")
                nc.vector.tensor_copy(rs[64:65, 0:512], ps_av_a[D:VW, :])
                nc.vector.tensor_copy(rs[64:65, 512:1024], ps_av_b[D:VW, :])
                nc.vector.tensor_copy(attT[0:64, osl], ps_av_a[0:64, :])
                tmb = rb_pool.tile([64, 512], F16, tag="tmb")
                nc.vector.tensor_copy(tmb, ps_av_b[0:64, :])
                bc_a = ps_w.tile([P, 512], F32, tag="w")
                nc.tensor.matmul(
                    bc_a[0:64, :], ones_bc[64:65, :], rs[64:65, 0:512],
                    start=True, stop=True, tile_position=(64, 0),
                    skip_group_check=True)
                bc_b = ps_w.tile([P, 512], F32, tag="w")
                nc.tensor.matmul(
                    bc_b[0:64, :], ones_bc[64:65, :], rs[64:65, 512:1024],
                    start=True, stop=True, tile_position=(64, 0),
                    skip_group_check=True)
                rbr = rs_pool.tile([64, 1024], F32, tag="rbr")
                nc.vector.tensor_copy(rbr[:, 0:512], bc_a[0:64, :])
                nc.vector.tensor_copy(rbr[:, 512:1024], bc_b[0:64, :])
                rb = rb_pool.tile([64, 1024], F32, tag="rb")
                nc.vector.reciprocal_approx_fast(out=rb, in_=rbr)
                nc.vector.tensor_mul(attT[0:64, osl],
                                     attT[0:64, osl], rb[:, 0:512])
                nc.vector.tensor_mul(tmb, tmb, rb[:, 512:1024])
                # head B's rows sit at partitions 0-63; shift to 64-127
                nc.sync.dma_start(out=attT[64:128, osl], in_=tmb)

        # ---- Projection tail: output tiles i = 0..3 (gated on the
        # last-processed pair-3 block, nb=0) ----
        for i in range(0, 4):
            for co in range(2):
                d_unit(i, co)


@functools.lru_cache(maxsize=1)
def build_nc():
    nc = bacc.Bacc("TRN2", target_bir_lowering=False, debug=False)
    xt_d = nc.dram_tensor("xt_local", [C, N], F16, kind="ExternalInput").ap()
    wq_d = nc.dram_tensor("wq", [PAIRS * P, CT * P], F16, kind="ExternalInput").ap()
    wk_d = nc.dram_tensor("wk", [PAIRS * P, CT * P], F16, kind="ExternalInput").ap()
    wv_d = nc.dram_tensor("wv", [P, CT * DCORE], F16, kind="ExternalInput").ap()
    wp_d = nc.dram_tensor("wp", [P, PAIRS * C], F16, kind="ExternalInput").ap()
    out_d = nc.dram_tensor("out_partial", [N, C], F32, kind="ExternalOutput").ap()
    with tile.TileContext(nc) as tc:
        _kernel_body(tc, out_d, xt_d, wq_d, wk_d, wv_d, wp_d)
    nc.compile()
    return nc


def make_in_maps(x, W_qkv, W_proj):
    """Stage inputs in the kernel's on-chip layouts (all fp16):
    - xt_local: x[b]^T, [C, N]
    - wq/wk: per-pair stationary slabs [PAIRS*P, CT*P]:
        row p*P+q, col cc*P+f  =  W[cc*P+q, p*P+f]
    - wv: [P, CT*DCORE]: row q, col cc*DCORE+f = W[cc*P+q, f]
    - wp: [P, PAIRS*C]:  row q, col dc*C+f     = W[dc*P+q, f]
    """
    def qk_stage(w):  # [C, DCORE] -> [PAIRS*P, CT*P]
        return np.ascontiguousarray(
            w.reshape(CT, P, PAIRS, P).transpose(2, 1, 0, 3)
            .reshape(PAIRS * P, CT * P).astype(np.float16))

    in_maps = []
    for core in range(NCORES):
        b, half = core // 2, core % 2
        h0 = half * HPC
        wq = W_qkv[:, 0 * C + h0 * D: 0 * C + h0 * D + DCORE]
        wk = W_qkv[:, 1 * C + h0 * D: 1 * C + h0 * D + DCORE]
        wv = W_qkv[:, 2 * C + h0 * D: 2 * C + h0 * D + DCORE]
        wp = W_proj[h0 * D: h0 * D + DCORE, :]
        in_maps.append({
            "xt_local": np.ascontiguousarray(x[b].T.astype(np.float16)),
            "wq": qk_stage(wq),
            "wk": qk_stage(wk),
            "wv": np.ascontiguousarray(
                wv.reshape(CT, P, DCORE).transpose(1, 0, 2)
                .reshape(P, CT * DCORE).astype(np.float16)),
            "wp": np.ascontiguousarray(
                wp.reshape(PAIRS, P, C).transpose(1, 0, 2)
                .reshape(P, PAIRS * C).astype(np.float16)),
        })
    return in_maps


def kernel(x, W_qkv, W_proj, b_proj, trace=False):
    x = np.asarray(x, dtype=np.float32)
    W_qkv = np.asarray(W_qkv, dtype=np.float32)
    W_proj = np.asarray(W_proj, dtype=np.float32)
    b_proj = np.asarray(b_proj, dtype=np.float32)

    nc = build_nc()
    in_maps = make_in_maps(x, W_qkv, W_proj)

    global LAST_RESULT
    res = run_bass_kernel_spmd(nc, in_maps, list(range(NCORES)), trace=trace)
    LAST_RESULT = res

    out = np.empty((B, N, C), dtype=np.float32)
    for b in range(B):
        out[b] = (res.results[2 * b]["out_partial"]
                  + res.results[2 * b + 1]["out_partial"]
                  + b_proj[None, :])
    return out
